# revision 2
# baseline (speedup 1.0000x reference)
"""Capsule routing pooling kernel for Trainium2 (8 NeuronCores, data parallel).

Math: the reference's softmax is over a singleton axis, so the routing
coefficients are identically 1.0 and the routing iterations never affect the
output.  The computation reduces to, per (b, c, 2x2 spatial tile):
    s   = sum of the four D=16 vectors in the tile
    sq  = sum_d s_d^2
    out = s * sq / ((1 + sq) * (sqrt(sq) + 1e-8))

Sharding: batch dim (16) split across 8 cores -> 2 batches/core.  Per core the
(2*64)=128 (b,c) pairs map onto the 128 SBUF partitions; each partition owns a
full 64x64x16 image.

v2 pipeline (bf16 datapath; rel-err budget is 2e-2, bf16 noise is ~4e-3):
  - loads are SWDGE (gpsimd) DMAs that cast f32 -> bf16 on the fly; HBM read
    bytes unchanged, SBUF write bytes halved, and every downstream DVE
    tensor_tensor op gets the 2x_1P bf16 perf mode
  - row-pair add then column-pair add on DVE in bf16 (2x)
  - per super-group: square on ACT (bf16 -> f32 PSUM), reduce over D + squash
    scale chain in f32 on DVE, scale cast to bf16, broadcast multiply (1x),
    bf16 store on the scalar HWDGE ring (loads own the SWDGE path)
  - output dram tensor is bf16 (half the store traffic); host upcasts
"""

import numpy as np

import concourse.bass as bass
import concourse.bacc as bacc
import concourse.tile as tile
from concourse import mybir
from concourse.bass_utils import run_bass_kernel_spmd

_B, _C, _H, _W, _D = 16, 64, 64, 64, 16
_NCORES = 8
_F32 = mybir.dt.float32
_BF16 = mybir.dt.bfloat16


def _kernel_body(tc, out_ap, in_ap, H, W, D, G=None, SG=None):
    nc = tc.nc
    P = 128
    nH, nW = H // 2, W // 2
    WD = W * D
    NL = nH // 2  # load units: 4 input rows (2 row-pairs, 16KB/partition) each

    inv4 = in_ap.rearrange("p (q four) w d -> p q (four w d)", four=4)
    inv2 = in_ap.rearrange("p (rp two) w d -> p rp (two w d)", two=2)
    outv = out_ap.rearrange("p y x d -> p y (x d)")

    # super-group schedule in row-pair units (even sizes only): small batches
    # at both ends (fast pipeline fill / short drain tail), big in the middle
    if nH >= 32:
        sched = [8] * ((nH - 16) // 8) + [8, 4, 2, 1, 1]
    elif nH >= 8:
        sched = [8] * (nH // 8)
    else:
        sched = [nH]
    assert sum(sched) == nH
    nsg_max = max(sched) * nW

    import contextlib

    with contextlib.ExitStack() as ctx:
        slabs = ctx.enter_context(tc.tile_pool(name="slabs", bufs=6))
        rpool = ctx.enter_context(tc.tile_pool(name="rpool", bufs=2))
        mid = ctx.enter_context(tc.tile_pool(name="mid", bufs=3))
        psum = ctx.enter_context(tc.tile_pool(name="psum", bufs=1, space="PSUM"))
        small = ctx.enter_context(tc.tile_pool(name="small", bufs=2))
        smallb = ctx.enter_context(tc.tile_pool(name="smallb", bufs=2))

        # one queued tail per super-group, emitted one SG late so the DVE
        # instruction stream never waits on ACT's square at SG boundaries
        pending = []

        def emit_front(sg, g0, fine=False):
            """loads + row-pair adds + column-pair adds for one super-group
            of `sg` row-pairs starting at output row g0.  fine=True loads one
            row-pair per DMA (1 MB) for fast pipeline fill."""
            s_sg = mid.tile([P, sg, nW, D], _BF16, tag="s_sg")
            for ci in range(0, sg, 4):
                cg = min(4, sg - ci)  # row-pairs this col-add batch
                r = rpool.tile([P, 4, nW, 2, D], _BF16, tag="r")
                for li in range(0, cg, 2):
                    if fine:
                        for q in range(min(2, cg - li)):
                            rp = g0 + ci + li + q
                            slab = slabs.tile([P, 1, 2, nW, 2, D], _BF16, tag="slab")
                            nc.gpsimd.dma_start(
                                out=slab[:],
                                in_=inv2[:, rp, :].rearrange(
                                    "p (two b) -> p two b", two=2
                                ),
                            )
                            nc.vector.tensor_add(
                                r[:, li + q : li + q + 1, :, :, :],
                                slab[:, :, 0, :, :, :],
                                slab[:, :, 1, :, :, :],
                            )
                        continue
                    t = (g0 + ci + li) // 2
                    slab = slabs.tile([P, 2, 2, nW, 2, D], _BF16, tag="slab")
                    nc.gpsimd.dma_start(
                        out=slab[:],
                        in_=inv4[:, t, :].rearrange(
                            "p (a two b) -> p a two b", a=2, two=2
                        ),
                    )
                    # row-pair sums for 2 row-pairs (DVE bf16 2x, FD=2048)
                    nc.vector.tensor_add(
                        r[:, li : li + 2, :, :, :],
                        slab[:, :, 0, :, :, :],
                        slab[:, :, 1, :, :, :],
                    )
                # column-pair add for cg row-pairs (DVE bf16 2x, FD=cg*512)
                nc.vector.tensor_add(
                    s_sg[:, ci : ci + cg, :, :],
                    r[:, 0:cg, :, 0, :],
                    r[:, 0:cg, :, 1, :],
                )
            return s_sg

        def emit_tail(sg, g0, s_sg):
            """square + reduce + squash scale + final multiply + store.
            ACT program order here keeps the next SG's square behind this
            SG's store, matching PSUM single-buffering."""
            nsg = sg * nW
            sv = s_sg[:].rearrange("p s x d -> p (s x) d")
            s2p = psum.tile([P, nsg, D], _F32, tag="s2p")
            nc.scalar.activation(s2p[:], sv, mybir.ActivationFunctionType.Square)
            ch = small.tile([P, nsg_max, 5], _F32, tag="ch")
            sq = ch[:, 0:nsg, 0:1]
            c1 = ch[:, 0:nsg, 1:2]
            a = ch[:, 0:nsg, 2:3]
            den = ch[:, 0:nsg, 3:4]
            rec = ch[:, 0:nsg, 4:5]
            scb = smallb.tile([P, nsg_max, 1], _BF16, tag="scb")
            sc = scb[:, 0:nsg, 0:1]
            nc.vector.tensor_reduce(
                sq, s2p[:], axis=mybir.AxisListType.X, op=mybir.AluOpType.add
            )
            # scale = sq / ((1 + sq) * sqrt(sq))   (1e-8 dropped: sq >= O(1)
            # for this distribution; relative effect <= 1e-6)
            nc.scalar.add(c1, sq, 1.0)
            nc.scalar.activation(a, sq, mybir.ActivationFunctionType.Sqrt)
            nc.vector.tensor_mul(den, c1, a)
            nc.vector.reciprocal_approx_fast(rec, den)
            nc.vector.tensor_mul(sc, sq, rec)
            # out = s * scale (broadcast over D), in place on s_sg
            nc.vector.tensor_mul(sv, sv, sc.to_broadcast((P, nsg, D)))
            nc.scalar.dma_start(
                out=outv[:, g0 : g0 + sg, :],
                in_=sv.rearrange("p n d -> p (n d)"),
            )

        def emit_tail2(t1, t2):
            """the last two tails, op-interleaved so ACT and DVE pipeline
            instead of ping-ponging through two serial chains."""
            (sg1, g01, s1), (sg2, g02, s2) = t1, t2
            n1, n2 = sg1 * nW, sg2 * nW
            sv1 = s1[:].rearrange("p s x d -> p (s x) d")
            sv2 = s2[:].rearrange("p s x d -> p (s x) d")

            def chain(nsg):
                ch = small.tile([P, nsg_max, 5], _F32, tag="ch")
                scb = smallb.tile([P, nsg_max, 1], _BF16, tag="scb")
                return [ch[:, 0:nsg, i : i + 1] for i in range(5)] + [
                    scb[:, 0:nsg, 0:1]
                ]

            sq1, c11, a1, den1, rec1, sc1 = chain(n1)
            sq2, c12, a2, den2, rec2, sc2 = chain(n2)
            p1 = psum.tile([P, n1, D], _F32, tag="s2p")
            nc.scalar.activation(p1[:], sv1, mybir.ActivationFunctionType.Square)
            nc.vector.tensor_reduce(
                sq1, p1[:], axis=mybir.AxisListType.X, op=mybir.AluOpType.add
            )
            p2 = psum.tile([P, n2, D], _F32, tag="s2p")
            nc.scalar.activation(p2[:], sv2, mybir.ActivationFunctionType.Square)
            nc.scalar.add(c11, sq1, 1.0)
            nc.scalar.activation(a1, sq1, mybir.ActivationFunctionType.Sqrt)
            nc.vector.tensor_mul(den1, c11, a1)
            nc.vector.reciprocal_approx_fast(rec1, den1)
            nc.vector.tensor_mul(sc1, sq1, rec1)
            nc.vector.tensor_reduce(
                sq2, p2[:], axis=mybir.AxisListType.X, op=mybir.AluOpType.add
            )
            nc.vector.tensor_mul(sv1, sv1, sc1.to_broadcast((P, n1, D)))
            nc.scalar.add(c12, sq2, 1.0)
            nc.scalar.activation(a2, sq2, mybir.ActivationFunctionType.Sqrt)
            nc.scalar.dma_start(
                out=outv[:, g01 : g01 + sg1, :], in_=sv1.rearrange("p n d -> p (n d)")
            )
            nc.vector.tensor_mul(den2, c12, a2)
            nc.vector.reciprocal_approx_fast(rec2, den2)
            nc.vector.tensor_mul(sc2, sq2, rec2)
            nc.vector.tensor_mul(sv2, sv2, sc2.to_broadcast((P, n2, D)))
            nc.scalar.dma_start(
                out=outv[:, g02 : g02 + sg2, :], in_=sv2.rearrange("p n d -> p (n d)")
            )

        g0 = 0
        last = len(sched) - 1
        for si, sg in enumerate(sched):
            fine = len(sched) > 2 and (si == 0 or si >= len(sched) - 2)
            front = emit_front(sg, g0, fine=fine)
            if pending and si < last:
                emit_tail(*pending.pop(0))
            pending.append((sg, g0, front))
            g0 += sg
        if len(pending) == 2:
            emit_tail2(pending[0], pending[1])
        else:
            for t in pending:
                emit_tail(*t)


def build_nc(H=_H, W=_W, D=_D, G=2):
    """Build and compile the per-core Bass program."""
    nc = bacc.Bacc("TRN2", target_bir_lowering=False, debug=False)
    inp = nc.dram_tensor("inp", [128, H, W, D], _F32, kind="ExternalInput").ap()
    out = nc.dram_tensor(
        "out", [128, H // 2, W // 2, D], _BF16, kind="ExternalOutput"
    ).ap()
    with tile.TileContext(nc) as tc:
        _kernel_body(tc, out, inp, H, W, D, G)
    nc.compile()
    return nc


_NC_CACHE = {}


def _get_nc():
    if "nc" not in _NC_CACHE:
        _NC_CACHE["nc"] = build_nc()
    return _NC_CACHE["nc"]


def kernel(inp, kernel_size=2, routing_iteration=3, _trace=False, _tmpdir=None):
    inp = np.asarray(inp, dtype=np.float32)
    assert int(kernel_size) == 2, "kernel compiled for kernel_size=2"
    assert inp.shape == (_B, _C, _H, _W, _D), inp.shape
    # routing_iteration is mathematically irrelevant (softmax over singleton
    # axis -> coefficients identically 1); any value >= 1 gives this output.

    nc = _get_nc()
    bpc = _B // _NCORES  # batches per core
    in_maps = [
        {"inp": np.ascontiguousarray(inp[i * bpc : (i + 1) * bpc]).reshape(128, _H, _W, _D)}
        for i in range(_NCORES)
    ]
    res = run_bass_kernel_spmd(
        nc, in_maps, core_ids=list(range(_NCORES)), trace=_trace, tmpdir=_tmpdir
    )
    out = np.empty((_B, _C, _H // 2, _W // 2, _D), dtype=np.float32)
    for i in range(_NCORES):
        out[i * bpc : (i + 1) * bpc] = (
            np.asarray(res.results[i]["out"])
            .astype(np.float32)
            .reshape(bpc, _C, _H // 2, _W // 2, _D)
        )
    if _trace:
        return out, res
    return out


# revision 4
# speedup vs baseline: 1.0298x; 1.0298x over previous
"""Capsule routing pooling kernel for Trainium2 (8 NeuronCores, data parallel).

Math: the reference's softmax is over a singleton axis, so the routing
coefficients are identically 1.0 and the routing iterations never affect the
output.  The computation reduces to, per (b, c, 2x2 spatial tile):
    s   = sum of the four D=16 vectors in the tile
    sq  = sum_d s_d^2
    out = s * sqrt(sq) / (1 + sq)

Sharding: batch dim (16) split across 8 cores -> 2 batches/core.  Per core the
(2*64)=128 (b,c) pairs map onto the 128 SBUF partitions; each partition owns a
full 64x64x16 image.

v3 pipeline (bf16 datapath; rel-err budget is 2e-2, bf16 noise is ~4e-3):
  - bulk loads are SWDGE (gpsimd) DMAs that cast f32 -> bf16 in the DMA
    datapath: HBM read bytes unchanged (the floor), SBUF writes halved, and
    DVE tensor_tensor ops get the bf16 2x_1P perf mode
  - the FIRST super-group loads f32 on the sync HWDGE ring instead: HWDGE
    issues ~6us before the gpsimd Q7 clears the preamble, hiding the SWDGE
    startup latency
  - super-groups of 4 row-pairs so the PSUM square tile (8KB/partition) can
    double-buffer: ACT squares SG i+1 while DVE reduces SG i - the tail
    chains pipeline instead of serializing on a single PSUM buffer
  - tails: ACT square -> DVE reduce over D -> scale = sqrt(sq) * 1/(1+sq)
    (ACT sqrt + DVE fast-reciprocal) -> DVE broadcast multiply -> bf16 store
    on the sync HWDGE ring; loads own the SWDGE path so neither blocks
  - output dram tensor is bf16 (half the store traffic); host upcasts
"""

import numpy as np

import concourse.bass as bass
import concourse.bacc as bacc
import concourse.tile as tile
from concourse import mybir
from concourse.bass_utils import run_bass_kernel_spmd

_B, _C, _H, _W, _D = 16, 64, 64, 64, 16
_NCORES = 8
_F32 = mybir.dt.float32
_BF16 = mybir.dt.bfloat16


def _kernel_body(tc, out_ap, in_ap, H, W, D):
    nc = tc.nc
    P = 128
    nH, nW = H // 2, W // 2

    inv4 = in_ap.rearrange("p (q four) w d -> p q (four w d)", four=4)
    inv2 = in_ap.rearrange("p (rp two) w d -> p rp (two w d)", two=2)
    outv = out_ap.rearrange("p y x d -> p y (x d)")

    # super-group schedule in row-pair units: 4s in the bulk (PSUM
    # double-buffering wants nsg <= 128), tiny SGs at the end for a short
    # post-last-load drain
    if nH >= 16:
        sched = [4] * ((nH - 4) // 4) + [2, 1, 1]
    else:
        sched = [nH]
    assert sum(sched) == nH
    nsg_max = max(sched) * nW

    import contextlib

    with contextlib.ExitStack() as ctx:
        slabs = ctx.enter_context(tc.tile_pool(name="slabs", bufs=6))
        rpool = ctx.enter_context(tc.tile_pool(name="rpool", bufs=2))
        mid = ctx.enter_context(tc.tile_pool(name="mid", bufs=3))
        psum = ctx.enter_context(tc.tile_pool(name="psum", bufs=2, space="PSUM"))
        small = ctx.enter_context(tc.tile_pool(name="small", bufs=2))
        smallb = ctx.enter_context(tc.tile_pool(name="smallb", bufs=2))

        # one queued tail per super-group, emitted one SG late so the DVE
        # instruction stream never waits on ACT's square at SG boundaries
        pending = []

        def emit_front(sg, g0, fine=False, warm=False):
            """loads + row-pair adds + column-pair adds for one super-group
            of `sg` row-pairs starting at output row g0.

            fine=True  -> one row-pair per DMA (fast fill / short drain)
            warm=True  -> f32 loads on the sync HWDGE ring (no Q7 preamble
                          wait); row-pair add casts f32 -> bf16 on DVE
            """
            s_sg = mid.tile([P, sg, nW, D], _BF16, tag="s_sg")
            for ci in range(0, sg, 4):
                cg = min(4, sg - ci)  # row-pairs this col-add batch
                r = rpool.tile([P, 4, nW, 2, D], _BF16, tag="r")
                for li in range(0, cg, 2):
                    if fine or warm:
                        for q in range(min(2, cg - li)):
                            rp = g0 + ci + li + q
                            dt = _F32 if warm else _BF16
                            slab = slabs.tile([P, 1, 2, nW, 2, D], dt, tag="slab")
                            eng = nc.sync if warm else nc.gpsimd
                            eng.dma_start(
                                out=slab[:],
                                in_=inv2[:, rp, :].rearrange(
                                    "p (two b) -> p two b", two=2
                                ),
                            )
                            nc.vector.tensor_add(
                                r[:, li + q : li + q + 1, :, :, :],
                                slab[:, :, 0, :, :, :],
                                slab[:, :, 1, :, :, :],
                            )
                        continue
                    t = (g0 + ci + li) // 2
                    slab = slabs.tile([P, 2, 2, nW, 2, D], _BF16, tag="slab")
                    nc.gpsimd.dma_start(
                        out=slab[:],
                        in_=inv4[:, t, :].rearrange(
                            "p (a two b) -> p a two b", a=2, two=2
                        ),
                    )
                    # row-pair sums for 2 row-pairs (DVE bf16 2x, FD=2048)
                    nc.vector.tensor_add(
                        r[:, li : li + 2, :, :, :],
                        slab[:, :, 0, :, :, :],
                        slab[:, :, 1, :, :, :],
                    )
                # column-pair add for cg row-pairs (DVE bf16 2x)
                nc.vector.tensor_add(
                    s_sg[:, ci : ci + cg, :, :],
                    r[:, 0:cg, :, 0, :],
                    r[:, 0:cg, :, 1, :],
                )
            return s_sg

        def chain_views(nsg):
            ch = small.tile([P, nsg_max, 3], _F32, tag="ch")
            scb = smallb.tile([P, nsg_max, 1], _BF16, tag="scb")
            sq = ch[:, 0:nsg, 0:1]
            a = ch[:, 0:nsg, 1:2]
            rec = ch[:, 0:nsg, 2:3]
            sc = scb[:, 0:nsg, 0:1]
            return sq, a, rec, sc

        def emit_tail(sg, g0, s_sg):
            """square + reduce + squash scale + final multiply + store."""
            nsg = sg * nW
            sv = s_sg[:].rearrange("p s x d -> p (s x) d")
            s2p = psum.tile([P, nsg, D], _F32, tag="s2p")
            nc.scalar.activation(s2p[:], sv, mybir.ActivationFunctionType.Square)
            sq, a, rec, sc = chain_views(nsg)
            nc.vector.tensor_reduce(
                sq, s2p[:], axis=mybir.AxisListType.X, op=mybir.AluOpType.add
            )
            # scale = sqrt(sq) / (1 + sq)   (1e-8 dropped: sq >= O(1) for
            # this distribution; relative effect <= 1e-6)
            nc.scalar.activation(a, sq, mybir.ActivationFunctionType.Sqrt)
            nc.scalar.add(rec, sq, 1.0)
            nc.vector.reciprocal_approx_fast(rec, rec)
            nc.vector.tensor_mul(sc, a, rec)
            # out = s * scale (broadcast over D), in place on s_sg
            nc.vector.tensor_mul(sv, sv, sc.to_broadcast((P, nsg, D)))
            nc.sync.dma_start(
                out=outv[:, g0 : g0 + sg, :],
                in_=sv.rearrange("p n d -> p (n d)"),
            )

        def emit_tail2(t1, t2):
            """the last two tails, op-interleaved so ACT and DVE pipeline
            instead of ping-ponging through two serial chains."""
            (sg1, g01, s1), (sg2, g02, s2) = t1, t2
            n1, n2 = sg1 * nW, sg2 * nW
            sv1 = s1[:].rearrange("p s x d -> p (s x) d")
            sv2 = s2[:].rearrange("p s x d -> p (s x) d")
            sq1, a1, rec1, sc1 = chain_views(n1)
            sq2, a2, rec2, sc2 = chain_views(n2)
            p1 = psum.tile([P, n1, D], _F32, tag="s2p")
            nc.scalar.activation(p1[:], sv1, mybir.ActivationFunctionType.Square)
            nc.vector.tensor_reduce(
                sq1, p1[:], axis=mybir.AxisListType.X, op=mybir.AluOpType.add
            )
            p2 = psum.tile([P, n2, D], _F32, tag="s2p")
            nc.scalar.activation(p2[:], sv2, mybir.ActivationFunctionType.Square)
            nc.scalar.activation(a1, sq1, mybir.ActivationFunctionType.Sqrt)
            nc.scalar.add(rec1, sq1, 1.0)
            nc.vector.reciprocal_approx_fast(rec1, rec1)
            nc.vector.tensor_mul(sc1, a1, rec1)
            nc.vector.tensor_reduce(
                sq2, p2[:], axis=mybir.AxisListType.X, op=mybir.AluOpType.add
            )
            nc.vector.tensor_mul(sv1, sv1, sc1.to_broadcast((P, n1, D)))
            nc.scalar.activation(a2, sq2, mybir.ActivationFunctionType.Sqrt)
            nc.scalar.add(rec2, sq2, 1.0)
            nc.sync.dma_start(
                out=outv[:, g01 : g01 + sg1, :], in_=sv1.rearrange("p n d -> p (n d)")
            )
            nc.vector.reciprocal_approx_fast(rec2, rec2)
            nc.vector.tensor_mul(sc2, a2, rec2)
            nc.vector.tensor_mul(sv2, sv2, sc2.to_broadcast((P, n2, D)))
            nc.sync.dma_start(
                out=outv[:, g02 : g02 + sg2, :], in_=sv2.rearrange("p n d -> p (n d)")
            )

        g0 = 0
        last = len(sched) - 1
        for si, sg in enumerate(sched):
            warm = si == 0
            fine = si >= len(sched) - 2
            front = emit_front(sg, g0, fine=fine, warm=warm)
            if pending and si < last:
                emit_tail(*pending.pop(0))
            pending.append((sg, g0, front))
            g0 += sg
        if len(pending) == 2:
            emit_tail2(pending[0], pending[1])
        else:
            for t in pending:
                emit_tail(*t)


def build_nc(H=_H, W=_W, D=_D):
    """Build and compile the per-core Bass program."""
    nc = bacc.Bacc("TRN2", target_bir_lowering=False, debug=False)
    inp = nc.dram_tensor("inp", [128, H, W, D], _F32, kind="ExternalInput").ap()
    out = nc.dram_tensor(
        "out", [128, H // 2, W // 2, D], _BF16, kind="ExternalOutput"
    ).ap()
    with tile.TileContext(nc) as tc:
        _kernel_body(tc, out, inp, H, W, D)
    nc.compile()
    return nc


_NC_CACHE = {}


def _get_nc():
    if "nc" not in _NC_CACHE:
        _NC_CACHE["nc"] = build_nc()
    return _NC_CACHE["nc"]


def kernel(inp, kernel_size=2, routing_iteration=3, _trace=False, _tmpdir=None):
    inp = np.asarray(inp, dtype=np.float32)
    assert int(kernel_size) == 2, "kernel compiled for kernel_size=2"
    assert inp.shape == (_B, _C, _H, _W, _D), inp.shape
    # routing_iteration is mathematically irrelevant (softmax over singleton
    # axis -> coefficients identically 1); any value >= 1 gives this output.

    nc = _get_nc()
    bpc = _B // _NCORES  # batches per core
    in_maps = [
        {"inp": np.ascontiguousarray(inp[i * bpc : (i + 1) * bpc]).reshape(128, _H, _W, _D)}
        for i in range(_NCORES)
    ]
    res = run_bass_kernel_spmd(
        nc, in_maps, core_ids=list(range(_NCORES)), trace=_trace, tmpdir=_tmpdir
    )
    out = np.empty((_B, _C, _H // 2, _W // 2, _D), dtype=np.float32)
    for i in range(_NCORES):
        out[i * bpc : (i + 1) * bpc] = (
            np.asarray(res.results[i]["out"])
            .astype(np.float32)
            .reshape(bpc, _C, _H // 2, _W // 2, _D)
        )
    if _trace:
        return out, res
    return out


# revision 5
# speedup vs baseline: 1.1605x; 1.1269x over previous
"""Capsule routing pooling kernel for Trainium2 (8 NeuronCores, data parallel).

Math: the reference's softmax is over a singleton axis, so the routing
coefficients are identically 1.0 and the routing iterations never affect the
output.  The computation reduces to, per (b, c, 2x2 spatial tile):
    s   = sum of the four D=16 vectors in the tile
    sq  = sum_d s_d^2
    out = s * sqrt(sq) / (1 + sq)

Sharding: batch dim (16) split across 8 cores -> 2 batches/core.  Per core the
(2*64)=128 (b,c) pairs map onto the 128 SBUF partitions; each partition owns a
full 64x64x16 image.

v4 pipeline (bf16 datapath; rel-err budget is 2e-2, bf16 noise is ~4e-3):
  - all loads are SWDGE (gpsimd) DMAs casting f32 -> bf16 in the DMA
    datapath: HBM read bytes unchanged (the floor, ~96us at measured
    333 GB/s), SBUF writes halved, and DVE tensor_tensor ops get the bf16
    2x_1P perf mode.  Loads are 2MB units (16KB/partition reads) to
    minimize Q7 descriptor-emission time, which gates the pipeline start.
  - pooled sums live in ONE persistent SBUF buffer (32KB/partition) instead
    of a recycled pool: fronts never block on store completion, so the load
    stream never stalls (v3's drain was fronts waiting on slow stores)
  - super-groups of 4 row-pairs so the PSUM square tile (8KB/partition)
    double-buffers: ACT squares SG i+1 while DVE reduces SG i
  - tails: ACT square -> DVE reduce over D -> scale = sqrt(sq) * 1/(1+sq)
    (ACT sqrt + DVE fast-reciprocal) -> DVE broadcast multiply in place
  - stores are decoupled from tails and batched to ~1MB (8KB/partition) on
    the otherwise-idle sync HWDGE ring; big store packets take few SDMA
    round-robin slots from the load queue
  - output dram tensor is bf16 (half the store traffic); host upcasts
"""

import numpy as np

import concourse.bass as bass
import concourse.bacc as bacc
import concourse.tile as tile
from concourse import mybir
from concourse.bass_utils import run_bass_kernel_spmd

_B, _C, _H, _W, _D = 16, 64, 64, 64, 16
_NCORES = 8
_F32 = mybir.dt.float32
_BF16 = mybir.dt.bfloat16


def _kernel_body(tc, out_ap, in_ap, H, W, D):
    nc = tc.nc
    P = 128
    nH, nW = H // 2, W // 2

    inv4 = in_ap.rearrange("p (q four) w d -> p q (four w d)", four=4)
    inv2 = in_ap.rearrange("p (rp two) w d -> p rp (two w d)", two=2)
    outv = out_ap.rearrange("p y x d -> p y (x d)")

    # super-group schedule in row-pair units: 4s in the bulk (PSUM
    # double-buffering wants nsg <= 128), tiny SGs at the end so the
    # post-last-load drain chain is short
    if nH >= 16:
        sched = [4] * ((nH - 4) // 4) + [2, 1, 1]
    else:
        sched = [nH]
    assert sum(sched) == nH
    nsg_max = max(sched) * nW

    import contextlib

    with contextlib.ExitStack() as ctx:
        slabs = ctx.enter_context(tc.tile_pool(name="slabs", bufs=8))
        rpool = ctx.enter_context(tc.tile_pool(name="rpool", bufs=3))
        sall_pool = ctx.enter_context(tc.tile_pool(name="sall", bufs=1))
        psum = ctx.enter_context(tc.tile_pool(name="psum", bufs=2, space="PSUM"))
        small = ctx.enter_context(tc.tile_pool(name="small", bufs=3))
        smallb = ctx.enter_context(tc.tile_pool(name="smallb", bufs=3))

        # persistent pooled-sum buffer for the whole image (bf16, 32KB/part)
        sall = sall_pool.tile([P, nH, nW, D], _BF16, tag="sall")

        def emit_front(sg, g0):
            """loads + row-pair adds + column-pair add for one super-group of
            `sg` row-pairs starting at output row g0; result lands in
            sall[:, g0:g0+sg]."""
            r = rpool.tile([P, 4, nW, 2, D], _BF16, tag="r")
            for li in range(0, sg, 2):
                if sg - li >= 2:
                    t = (g0 + li) // 2
                    slab = slabs.tile([P, 2, 2, nW, 2, D], _BF16, tag="slab")
                    nc.gpsimd.dma_start(
                        out=slab[:],
                        in_=inv4[:, t, :].rearrange(
                            "p (a two b) -> p a two b", a=2, two=2
                        ),
                    )
                    # row-pair sums for 2 row-pairs (DVE bf16 2x, FD=2048)
                    nc.vector.tensor_add(
                        r[:, li : li + 2, :, :, :],
                        slab[:, :, 0, :, :, :],
                        slab[:, :, 1, :, :, :],
                    )
                else:
                    rp = g0 + li
                    slab = slabs.tile([P, 1, 2, nW, 2, D], _BF16, tag="slab")
                    nc.gpsimd.dma_start(
                        out=slab[:],
                        in_=inv2[:, rp, :].rearrange("p (two b) -> p two b", two=2),
                    )
                    nc.vector.tensor_add(
                        r[:, li : li + 1, :, :, :],
                        slab[:, :, 0, :, :, :],
                        slab[:, :, 1, :, :, :],
                    )
            # column-pair add (DVE bf16 2x)
            nc.vector.tensor_add(
                sall[:, g0 : g0 + sg, :, :],
                r[:, 0:sg, :, 0, :],
                r[:, 0:sg, :, 1, :],
            )

        def chain_views(nsg):
            ch = small.tile([P, nsg_max, 3], _F32, tag="ch")
            scb = smallb.tile([P, nsg_max, 1], _BF16, tag="scb")
            sq = ch[:, 0:nsg, 0:1]
            a = ch[:, 0:nsg, 1:2]
            rec = ch[:, 0:nsg, 2:3]
            sc = scb[:, 0:nsg, 0:1]
            return sq, a, rec, sc

        def emit_tail(sg, g0):
            """square + reduce + squash scale + in-place broadcast multiply
            for rows [g0, g0+sg) of sall."""
            nsg = sg * nW
            sv = sall[:, g0 : g0 + sg, :, :].rearrange("p s x d -> p (s x) d")
            s2p = psum.tile([P, nsg_max, D], _F32, tag="s2p")
            nc.scalar.activation(
                s2p[:, 0:nsg, :], sv, mybir.ActivationFunctionType.Square
            )
            sq, a, rec, sc = chain_views(nsg)
            nc.vector.tensor_reduce(
                sq, s2p[:, 0:nsg, :], axis=mybir.AxisListType.X, op=mybir.AluOpType.add
            )
            # scale = sqrt(sq) / (1 + sq)   (1e-8 dropped: sq >= O(1) for
            # this distribution; relative effect <= 1e-6)
            nc.scalar.activation(a, sq, mybir.ActivationFunctionType.Sqrt)
            nc.scalar.add(rec, sq, 1.0)
            nc.vector.reciprocal_approx_fast(rec, rec)
            nc.vector.tensor_mul(sc, a, rec)
            # out = s * scale (broadcast over D), in place on sall
            nc.vector.tensor_mul(sv, sv, sc.to_broadcast((P, nsg, D)))

        def emit_store(y0, y1):
            nc.sync.dma_start(
                out=outv[:, y0:y1, :],
                in_=sall[:, y0:y1, :, :].rearrange("p s x d -> p (s x d)"),
            )

        def emit_tail2(t1, t2):
            """the last two tails, op-interleaved so ACT and DVE pipeline
            instead of ping-ponging through two serial chains."""
            (sg1, g01), (sg2, g02) = t1, t2
            n1, n2 = sg1 * nW, sg2 * nW
            sv1 = sall[:, g01 : g01 + sg1, :, :].rearrange("p s x d -> p (s x) d")
            sv2 = sall[:, g02 : g02 + sg2, :, :].rearrange("p s x d -> p (s x) d")
            sq1, a1, rec1, sc1 = chain_views(n1)
            sq2, a2, rec2, sc2 = chain_views(n2)
            p1 = psum.tile([P, nsg_max, D], _F32, tag="s2p")
            nc.scalar.activation(
                p1[:, 0:n1, :], sv1, mybir.ActivationFunctionType.Square
            )
            nc.vector.tensor_reduce(
                sq1, p1[:, 0:n1, :], axis=mybir.AxisListType.X, op=mybir.AluOpType.add
            )
            p2 = psum.tile([P, nsg_max, D], _F32, tag="s2p")
            nc.scalar.activation(
                p2[:, 0:n2, :], sv2, mybir.ActivationFunctionType.Square
            )
            nc.scalar.activation(a1, sq1, mybir.ActivationFunctionType.Sqrt)
            nc.scalar.add(rec1, sq1, 1.0)
            nc.vector.reciprocal_approx_fast(rec1, rec1)
            nc.vector.tensor_mul(sc1, a1, rec1)
            nc.vector.tensor_reduce(
                sq2, p2[:, 0:n2, :], axis=mybir.AxisListType.X, op=mybir.AluOpType.add
            )
            nc.vector.tensor_mul(sv1, sv1, sc1.to_broadcast((P, n1, D)))
            nc.scalar.activation(a2, sq2, mybir.ActivationFunctionType.Sqrt)
            nc.scalar.add(rec2, sq2, 1.0)
            nc.vector.reciprocal_approx_fast(rec2, rec2)
            nc.vector.tensor_mul(sc2, a2, rec2)
            nc.vector.tensor_mul(sv2, sv2, sc2.to_broadcast((P, n2, D)))

        g0 = 0
        last = len(sched) - 1
        pending = []  # (sg, g0) awaiting tail
        done_tails = []  # (sg, g0) tails emitted, store pending
        stored_to = 0

        def flush_store(upto_rows=2**30, min_rows=8):
            nonlocal stored_to
            done_rows = sum(sg for sg, _ in done_tails)
            if done_rows >= min_rows or (done_tails and min_rows == 0):
                y1 = min(stored_to + done_rows, upto_rows)
                if y1 > stored_to:
                    emit_store(stored_to, y1)
                    stored_to = y1
                    done_tails.clear()

        for si, sg in enumerate(sched):
            emit_front(sg, g0)
            if pending and si < last:
                tsg, tg0 = pending.pop(0)
                emit_tail(tsg, tg0)
                done_tails.append((tsg, tg0))
                flush_store()
            pending.append((sg, g0))
            g0 += sg
        if len(pending) == 2:
            emit_tail2(pending[0], pending[1])
            done_tails.extend(pending)
            pending.clear()
        else:
            for t in pending:
                emit_tail(*t)
                done_tails.append(t)
            pending.clear()
        flush_store(min_rows=0)
        assert stored_to == nH


def build_nc(H=_H, W=_W, D=_D):
    """Build and compile the per-core Bass program."""
    nc = bacc.Bacc("TRN2", target_bir_lowering=False, debug=False)
    inp = nc.dram_tensor("inp", [128, H, W, D], _F32, kind="ExternalInput").ap()
    out = nc.dram_tensor(
        "out", [128, H // 2, W // 2, D], _BF16, kind="ExternalOutput"
    ).ap()
    with tile.TileContext(nc) as tc:
        _kernel_body(tc, out, inp, H, W, D)
    nc.compile()
    return nc


_NC_CACHE = {}


def _get_nc():
    if "nc" not in _NC_CACHE:
        _NC_CACHE["nc"] = build_nc()
    return _NC_CACHE["nc"]


def kernel(inp, kernel_size=2, routing_iteration=3, _trace=False, _tmpdir=None):
    inp = np.asarray(inp, dtype=np.float32)
    assert int(kernel_size) == 2, "kernel compiled for kernel_size=2"
    assert inp.shape == (_B, _C, _H, _W, _D), inp.shape
    # routing_iteration is mathematically irrelevant (softmax over singleton
    # axis -> coefficients identically 1); any value >= 1 gives this output.

    nc = _get_nc()
    bpc = _B // _NCORES  # batches per core
    in_maps = [
        {"inp": np.ascontiguousarray(inp[i * bpc : (i + 1) * bpc]).reshape(128, _H, _W, _D)}
        for i in range(_NCORES)
    ]
    res = run_bass_kernel_spmd(
        nc, in_maps, core_ids=list(range(_NCORES)), trace=_trace, tmpdir=_tmpdir
    )
    out = np.empty((_B, _C, _H // 2, _W // 2, _D), dtype=np.float32)
    for i in range(_NCORES):
        out[i * bpc : (i + 1) * bpc] = (
            np.asarray(res.results[i]["out"])
            .astype(np.float32)
            .reshape(bpc, _C, _H // 2, _W // 2, _D)
        )
    if _trace:
        return out, res
    return out
